# revision 2
# baseline (speedup 1.0000x reference)
"""Trainium2 Bass kernel for nn_AddInterpolant (dense MLP + JVP interpolant).

Data-parallel over 8 NeuronCores: batch 65536 is split into 8 shards of
8192 rows; the small MLP weights are replicated.  Per core the kernel
computes, for z = concat(x0, x1, t):

    fnn    = W4.(relu(W3.(relu(W2.(relu(W1.z + b1)) + b2)) + b3)) + b4
    dt_fnn = d fnn / dt   (forward-mode JVP with one-hot tangent on t)
    xt     = (1-t) x0 + t x1 + t (1-t) fnn
    dt_xt  = x1 - x0 + (1-2t) fnn + t (1-t) dt_fnn

Layout: activations are kept transposed (features on SBUF partitions,
batch on the free axis) so every layer is a plain accumulation of
128x128 weight-block matmuls; inputs/outputs are transposed on the PE
via identity matmuls.  Matmuls run in float32r (full-rate fp32 path).
"""

import sys

for _p in ("/opt/trn_rl_repo",):
    if _p not in sys.path:
        sys.path.insert(0, _p)

import numpy as np

import concourse.mybir as mybir
import concourse.tile as tile
from concourse import bacc
from concourse.bass import ds
from concourse.bass_utils import run_bass_kernel_spmd
from concourse.masks import make_identity

P = 128
D = 256  # state dim
H = 1024  # hidden dim
B = 65536  # global batch
NCORES = 8
BL = B // NCORES  # rows per core
S = 512  # batch columns per stripe
NSTRIPES = BL // S
HC = H // P  # 8 hidden chunks
DC = D // P  # 2 state chunks

F32 = mybir.dt.float32
F32R = mybir.dt.float32r
RELU = mybir.ActivationFunctionType.Relu
IDENT = mybir.ActivationFunctionType.Identity
SIGN = mybir.ActivationFunctionType.Sign
GT = mybir.AluOpType.is_gt
MULT = mybir.AluOpType.mult
ADD = mybir.AluOpType.add
SUB = mybir.AluOpType.subtract
MAX = mybir.AluOpType.max

_nc_cache = None


def _r(ap):
    return ap.bitcast(F32R)


def build():
    nc = bacc.Bacc(None)

    x0e = nc.declare_dram_parameter("x0", [BL, D], F32, isOutput=False)
    x1e = nc.declare_dram_parameter("x1", [BL, D], F32, isOutput=False)
    te = nc.declare_dram_parameter("t", [BL, 1], F32, isOutput=False)
    W1e = nc.declare_dram_parameter("W1", [2 * D + 1, H], F32, isOutput=False)
    b1e = nc.declare_dram_parameter("b1", [H], F32, isOutput=False)
    W2e = nc.declare_dram_parameter("W2", [H, H], F32, isOutput=False)
    b2e = nc.declare_dram_parameter("b2", [H], F32, isOutput=False)
    W3e = nc.declare_dram_parameter("W3", [H, H], F32, isOutput=False)
    b3e = nc.declare_dram_parameter("b3", [H], F32, isOutput=False)
    W4e = nc.declare_dram_parameter("W4", [H, D], F32, isOutput=False)
    b4e = nc.declare_dram_parameter("b4", [D], F32, isOutput=False)
    xte = nc.declare_dram_parameter("xt", [BL, D], F32, isOutput=True)
    dte = nc.declare_dram_parameter("dt_xt", [BL, D], F32, isOutput=True)

    with tile.TileContext(nc) as tc:
        with (
            tc.tile_pool(name="const", bufs=1) as cp,
            tc.tile_pool(name="z", bufs=1) as zp,
            tc.tile_pool(name="acts", bufs=1) as hp,
            tc.tile_pool(name="outs", bufs=1) as fp,
            tc.tile_pool(name="nat", bufs=2) as npl,
            tc.tile_pool(name="small", bufs=2) as sp,
            tc.tile_pool(name="mm", bufs=2, space="PSUM") as mmp,
            tc.tile_pool(name="tps", bufs=3, space="PSUM") as tpp,
        ):
            # ---- constants / weights (f32r via direct DMA) ----
            w1s = cp.tile([P, 4, H], F32R)
            nc.sync.dma_start(
                w1s[:], _r(W1e[0 : 2 * D].rearrange("(o p) n -> p o n", p=P))
            )
            w1row = cp.tile([1, H], F32R)
            nc.sync.dma_start(w1row[:], _r(W1e[2 * D : 2 * D + 1, :]))
            w2s = cp.tile([P, HC, H], F32R)
            nc.sync.dma_start(w2s[:], _r(W2e.rearrange("(o p) n -> p o n", p=P)))
            w3s = cp.tile([P, HC, H], F32R)
            nc.sync.dma_start(w3s[:], _r(W3e.rearrange("(o p) n -> p o n", p=P)))
            w4s = cp.tile([P, HC, D], F32R)
            nc.sync.dma_start(w4s[:], _r(W4e.rearrange("(o p) n -> p o n", p=P)))
            w1rp = cp.tile([P, HC], F32)
            nc.sync.dma_start(w1rp[:], W1e[2 * D, :].rearrange("(o p) -> p o", p=P))
            b1p = cp.tile([P, HC], F32)
            nc.sync.dma_start(b1p[:], b1e.rearrange("(o p) -> p o", p=P))
            b2p = cp.tile([P, HC], F32)
            nc.sync.dma_start(b2p[:], b2e.rearrange("(o p) -> p o", p=P))
            b3p = cp.tile([P, HC], F32)
            nc.sync.dma_start(b3p[:], b3e.rearrange("(o p) -> p o", p=P))
            b4p = cp.tile([P, DC], F32)
            nc.sync.dma_start(b4p[:], b4e.rearrange("(o p) -> p o", p=P))
            ident = cp.tile([P, P], F32)
            make_identity(nc, ident)

            for s in range(NSTRIPES):
                row0 = s * S
                # ---- stripe inputs ----
                nat0 = npl.tile([P, 4, D], F32, tag="nat0")
                nc.sync.dma_start(
                    nat0[:], x0e[ds(row0, S), :].rearrange("(c p) f -> p c f", p=P)
                )
                nat1 = npl.tile([P, 4, D], F32, tag="nat1")
                nc.sync.dma_start(
                    nat1[:], x1e[ds(row0, S), :].rearrange("(c p) f -> p c f", p=P)
                )
                trow = sp.tile([1, S], F32R, tag="trow")
                nc.sync.dma_start(
                    trow[:], _r(te[ds(row0, S), 0:1].rearrange("b one -> one b"))
                )
                tnat = sp.tile([P, 4], F32, tag="tnat")
                nc.sync.dma_start(
                    tnat[:], te[ds(row0, S), 0].rearrange("(c p) -> p c", p=P)
                )

                # ---- transpose inputs into zT chunks ----
                zT = zp.tile([P, 4, S], F32R, tag="zT")
                for k in range(4):
                    src = nat0 if k < 2 else nat1
                    fc = k % 2
                    ps = tpp.tile([P, S], F32, tag="tps")
                    for c in range(4):
                        nc.tensor.transpose(
                            ps[:, ds(c * P, P)],
                            src[:, c, ds(fc * P, P)],
                            ident[:],
                        )
                    nc.vector.tensor_copy(zT[:, k, :], ps[:])

                # ---- layer 1 ----
                h1 = hp.tile([P, HC, S], F32R, tag="hA")
                dh1 = hp.tile([P, HC, S], F32R, tag="dhA")
                for m in range(HC):
                    psf = mmp.tile([P, S], F32, tag="mmf")
                    for k in range(4):
                        nc.tensor.matmul(
                            psf[:],
                            w1s[:, k, ds(m * P, P)],
                            zT[:, k, :],
                            start=(k == 0),
                            stop=False,
                        )
                    nc.tensor.matmul(
                        psf[:],
                        w1row[0:1, ds(m * P, P)],
                        trow[0:1, :],
                        start=False,
                        stop=True,
                    )
                    nc.scalar.activation(
                        h1[:, m, :], psf[:], RELU, bias=b1p[:, m : m + 1]
                    )
                    nc.vector.tensor_scalar(
                        dh1[:, m, :], h1[:, m, :], 0.0, w1rp[:, m : m + 1], GT, MULT
                    )

                # ---- layers 2 and 3 ----
                hprev, dhprev = h1, dh1
                for li, (ws, bp) in enumerate(((w2s, b2p), (w3s, b3p))):
                    hn = hp.tile([P, HC, S], F32R, tag="hB" if li == 0 else "hA")
                    dhn = hp.tile([P, HC, S], F32R, tag="dhB" if li == 0 else "dhA")
                    for m in range(HC):
                        psf = mmp.tile([P, S], F32, tag="mmf")
                        pst = mmp.tile([P, S], F32, tag="mmt")
                        for k in range(HC):
                            nc.tensor.matmul(
                                psf[:],
                                ws[:, k, ds(m * P, P)],
                                hprev[:, k, :],
                                start=(k == 0),
                                stop=(k == HC - 1),
                            )
                            nc.tensor.matmul(
                                pst[:],
                                ws[:, k, ds(m * P, P)],
                                dhprev[:, k, :],
                                start=(k == 0),
                                stop=(k == HC - 1),
                            )
                        # relu epilogue on DVE: (psum + b) max 0
                        nc.vector.tensor_scalar(
                            hn[:, m, :], psf[:], bp[:, m : m + 1], 0.0, ADD, MAX
                        )
                        # tangent mask: sign(h) in {0,1} on ACT, then mult on DVE
                        msk = sp.tile([P, S], F32, tag="mask")
                        nc.scalar.activation(msk[:], hn[:, m, :], SIGN)
                        nc.vector.tensor_tensor(dhn[:, m, :], msk[:], pst[:], MULT)
                    hprev, dhprev = hn, dhn

                # ---- layer 4 (no relu) ----
                fnnT = fp.tile([P, DC, S], F32, tag="fnnT")
                dfnnT = fp.tile([P, DC, S], F32, tag="dfnnT")
                for m in range(DC):
                    psf = mmp.tile([P, S], F32, tag="mmf")
                    pst = mmp.tile([P, S], F32, tag="mmt")
                    for k in range(HC):
                        nc.tensor.matmul(
                            psf[:],
                            w4s[:, k, ds(m * P, P)],
                            hprev[:, k, :],
                            start=(k == 0),
                            stop=(k == HC - 1),
                        )
                        nc.tensor.matmul(
                            pst[:],
                            w4s[:, k, ds(m * P, P)],
                            dhprev[:, k, :],
                            start=(k == 0),
                            stop=(k == HC - 1),
                        )
                    nc.scalar.activation(
                        fnnT[:, m, :], psf[:], IDENT, bias=b4p[:, m : m + 1]
                    )
                    nc.scalar.copy(dfnnT[:, m, :], pst[:])

                # ---- per-stripe t-derived scalars ----
                tsq = sp.tile([P, 4], F32, tag="tsq")
                nc.vector.tensor_tensor(tsq[:], tnat[:], tnat[:], MULT)
                a_ = sp.tile([P, 4], F32, tag="a_")
                nc.vector.tensor_tensor(a_[:], tnat[:], tsq[:], SUB)
                omt = sp.tile([P, 4], F32, tag="omt")
                nc.vector.tensor_scalar(omt[:], tnat[:], -1.0, 1.0, MULT, ADD)
                om2t = sp.tile([P, 4], F32, tag="om2t")
                nc.vector.tensor_scalar(om2t[:], tnat[:], -2.0, 1.0, MULT, ADD)

                # ---- transpose fnn/dfnn back to natural + combine ----
                dt_nat = fp.tile([P, 4, D], F32, tag="dt_nat")
                for cp_i in range(2):
                    psF = tpp.tile([P, 2, D], F32, tag="tps")
                    psD = tpp.tile([P, 2, D], F32, tag="tps")
                    for ci in range(2):
                        c = 2 * cp_i + ci
                        for fc in range(DC):
                            nc.tensor.transpose(
                                psF[:, ci, ds(fc * P, P)],
                                fnnT[:, fc, ds(c * P, P)],
                                ident[:],
                            )
                            nc.tensor.transpose(
                                psD[:, ci, ds(fc * P, P)],
                                dfnnT[:, fc, ds(c * P, P)],
                                ident[:],
                            )
                    for ci in range(2):
                        c = 2 * cp_i + ci
                        tc_ = tnat[:, c : c + 1]
                        # dt = x1 - x0 (must read x0 before xt overwrites nat0)
                        nc.vector.tensor_tensor(
                            dt_nat[:, c, :], nat1[:, c, :], nat0[:, c, :], SUB
                        )
                        tm1 = sp.tile([P, D], F32, tag="tmp")
                        nc.scalar.mul(tm1[:], nat1[:, c, :], tc_)  # t*x1
                        # xt partial: x0*(1-t) in place
                        nc.vector.tensor_scalar(
                            nat0[:, c, :], nat0[:, c, :], omt[:, c : c + 1], None, MULT
                        )
                        tm2 = sp.tile([P, D], F32, tag="tmp")
                        nc.scalar.mul(tm2[:], psF[:, ci, :], a_[:, c : c + 1])
                        nc.vector.tensor_tensor(
                            nat0[:, c, :], nat0[:, c, :], tm1[:], ADD
                        )
                        nc.vector.tensor_tensor(
                            nat0[:, c, :], nat0[:, c, :], tm2[:], ADD
                        )
                        tm3 = sp.tile([P, D], F32, tag="tmp")
                        nc.scalar.mul(tm3[:], psF[:, ci, :], om2t[:, c : c + 1])
                        nc.vector.tensor_tensor(
                            dt_nat[:, c, :], dt_nat[:, c, :], tm3[:], ADD
                        )
                        tm4 = sp.tile([P, D], F32, tag="tmp")
                        nc.scalar.mul(tm4[:], psD[:, ci, :], a_[:, c : c + 1])
                        nc.vector.tensor_tensor(
                            dt_nat[:, c, :], dt_nat[:, c, :], tm4[:], ADD
                        )

                nc.sync.dma_start(
                    xte[ds(row0, S), :].rearrange("(c p) f -> p c f", p=P), nat0[:]
                )
                nc.sync.dma_start(
                    dte[ds(row0, S), :].rearrange("(c p) f -> p c f", p=P), dt_nat[:]
                )

    nc.compile()
    return nc


def _get_nc():
    global _nc_cache
    if _nc_cache is None:
        _nc_cache = build()
    return _nc_cache


def kernel(x0, x1, t, W1, b1, W2, b2, W3, b3, W4, b4, trace=False, **trace_kwargs):
    nc = _get_nc()
    reps = {
        "W1": np.ascontiguousarray(W1, np.float32),
        "b1": np.ascontiguousarray(b1, np.float32),
        "W2": np.ascontiguousarray(W2, np.float32),
        "b2": np.ascontiguousarray(b2, np.float32),
        "W3": np.ascontiguousarray(W3, np.float32),
        "b3": np.ascontiguousarray(b3, np.float32),
        "W4": np.ascontiguousarray(W4, np.float32),
        "b4": np.ascontiguousarray(b4, np.float32),
    }
    in_maps = []
    for c in range(NCORES):
        sl = slice(c * BL, (c + 1) * BL)
        in_maps.append(
            {
                "x0": np.ascontiguousarray(x0[sl], np.float32),
                "x1": np.ascontiguousarray(x1[sl], np.float32),
                "t": np.ascontiguousarray(t[sl], np.float32),
                **reps,
            }
        )
    res = run_bass_kernel_spmd(
        nc, in_maps, list(range(NCORES)), trace=trace, **trace_kwargs
    )
    xt = np.concatenate([res.results[c]["xt"] for c in range(NCORES)], axis=0)
    dt_xt = np.concatenate([res.results[c]["dt_xt"] for c in range(NCORES)], axis=0)
    if trace:
        kernel.last_result = res
    return (xt, dt_xt)


# revision 9
# speedup vs baseline: 1.1218x; 1.1218x over previous
"""Trainium2 Bass kernel for nn_AddInterpolant (dense MLP + JVP interpolant).

Data-parallel over 8 NeuronCores: batch 65536 is split into 8 shards of
8192 rows; the small MLP weights are replicated.  Per core the kernel
computes, for z = concat(x0, x1, t):

    fnn    = W4.(relu(W3.(relu(W2.(relu(W1.z + b1)) + b2)) + b3)) + b4
    dt_fnn = d fnn / dt   (forward-mode JVP with one-hot tangent on t)
    xt     = (1-t) x0 + t x1 + t (1-t) fnn
    dt_xt  = x1 - x0 + (1-2t) fnn + t (1-t) dt_fnn

Layout: activations are kept transposed (features on SBUF partitions,
batch on the free axis) so every layer is a plain accumulation of
128x128 weight-block matmuls; inputs/outputs are transposed on the PE
via identity matmuls.  Matmuls run in float32r (full-rate fp32 path).
"""

import sys

for _p in ("/opt/trn_rl_repo",):
    if _p not in sys.path:
        sys.path.insert(0, _p)

import numpy as np

import concourse.mybir as mybir
import concourse.tile as tile
from concourse import bacc
from concourse.bass import ds
from concourse.bass_utils import run_bass_kernel_spmd
from concourse.masks import make_identity

P = 128
D = 256  # state dim
H = 1024  # hidden dim
B = 65536  # global batch
NCORES = 8
BL = B // NCORES  # rows per core
S = 512  # batch columns per stripe
NSTRIPES = BL // S
HC = H // P  # 8 hidden chunks
DC = D // P  # 2 state chunks

F32 = mybir.dt.float32
F32R = mybir.dt.float32r
RELU = mybir.ActivationFunctionType.Relu
IDENT = mybir.ActivationFunctionType.Identity
SIGN = mybir.ActivationFunctionType.Sign
GT = mybir.AluOpType.is_gt
MULT = mybir.AluOpType.mult
ADD = mybir.AluOpType.add
SUB = mybir.AluOpType.subtract
MAX = mybir.AluOpType.max

_nc_cache = None


def _r(ap):
    return ap.bitcast(F32R)


def build():
    nc = bacc.Bacc(None)

    x0e = nc.declare_dram_parameter("x0", [BL, D], F32, isOutput=False)
    x1e = nc.declare_dram_parameter("x1", [BL, D], F32, isOutput=False)
    te = nc.declare_dram_parameter("t", [BL, 1], F32, isOutput=False)
    W1e = nc.declare_dram_parameter("W1", [2 * D + 1, H], F32, isOutput=False)
    b1e = nc.declare_dram_parameter("b1", [H], F32, isOutput=False)
    W2e = nc.declare_dram_parameter("W2", [H, H], F32, isOutput=False)
    b2e = nc.declare_dram_parameter("b2", [H], F32, isOutput=False)
    W3e = nc.declare_dram_parameter("W3", [H, H], F32, isOutput=False)
    b3e = nc.declare_dram_parameter("b3", [H], F32, isOutput=False)
    W4e = nc.declare_dram_parameter("W4", [H, D], F32, isOutput=False)
    b4e = nc.declare_dram_parameter("b4", [D], F32, isOutput=False)
    xte = nc.declare_dram_parameter("xt", [BL, D], F32, isOutput=True)
    dte = nc.declare_dram_parameter("dt_xt", [BL, D], F32, isOutput=True)

    with tile.TileContext(nc) as tc:
        with (
            tc.tile_pool(name="const", bufs=1) as cp,
            tc.tile_pool(name="z", bufs=1) as zp,
            tc.tile_pool(name="acts", bufs=1) as hp,
            tc.tile_pool(name="outs", bufs=1) as fp,
            tc.tile_pool(name="nat", bufs=2) as npl,
            tc.tile_pool(name="small", bufs=2) as sp,
            tc.tile_pool(name="mm", bufs=2, space="PSUM") as mmp,
            tc.tile_pool(name="tps", bufs=3, space="PSUM") as tpp,
        ):
            # ---- constants / weights (f32r via direct DMA) ----
            w1s = cp.tile([P, 4, H], F32R)
            nc.sync.dma_start(
                w1s[:], _r(W1e[0 : 2 * D].rearrange("(o p) n -> p o n", p=P))
            )
            w2s = cp.tile([P, HC, H], F32R)
            nc.sync.dma_start(w2s[:], _r(W2e.rearrange("(o p) n -> p o n", p=P)))
            w3s = cp.tile([P, HC, H], F32R)
            nc.sync.dma_start(w3s[:], _r(W3e.rearrange("(o p) n -> p o n", p=P)))
            w4s = cp.tile([P, HC, D], F32R)
            nc.sync.dma_start(w4s[:], _r(W4e.rearrange("(o p) n -> p o n", p=P)))
            w1rp = cp.tile([P, HC], F32)
            nc.sync.dma_start(w1rp[:], W1e[2 * D, :].rearrange("(o p) -> p o", p=P))
            b1p = cp.tile([P, HC], F32)
            nc.sync.dma_start(b1p[:], b1e.rearrange("(o p) -> p o", p=P))
            b2p = cp.tile([P, HC], F32)
            nc.sync.dma_start(b2p[:], b2e.rearrange("(o p) -> p o", p=P))
            b3p = cp.tile([P, HC], F32)
            nc.sync.dma_start(b3p[:], b3e.rearrange("(o p) -> p o", p=P))
            b4p = cp.tile([P, DC], F32)
            nc.sync.dma_start(b4p[:], b4e.rearrange("(o p) -> p o", p=P))
            ident = cp.tile([P, P], F32)
            make_identity(nc, ident)
            ident_r = cp.tile([P, P], F32R)
            nc.vector.tensor_copy(ident_r[:], ident[:])
            # padded "t chunk": Z5 row0 = t (per stripe), rest 0; W15 row0 = W1[512]
            zstage = fp.tile([P, 4, D], F32, tag="dt_nat", name="zstage")
            nc.vector.memset(zstage[:], 0.0)
            z5 = cp.tile([P, S], F32R)
            nc.vector.tensor_copy(z5[:], zstage[:, 0:2, :].rearrange("p a b -> p (a b)"))
            w15 = cp.tile([P, H], F32R)
            nc.vector.tensor_copy(w15[:], zstage[:].rearrange("p a b -> p (a b)"))
            nc.sync.dma_start(w15[0:1, :], _r(W1e[2 * D : 2 * D + 1, :]))

            for s in range(NSTRIPES):
                row0 = s * S
                # ---- stripe inputs ----
                nat0 = npl.tile([P, 4, D], F32, tag="nat0")
                nc.sync.dma_start(
                    _r(nat0[:]), _r(x0e[ds(row0, S), :].rearrange("(c p) f -> p c f", p=P))
                )
                nat1 = npl.tile([P, 4, D], F32, tag="nat1")
                nc.sync.dma_start(
                    _r(nat1[:]), _r(x1e[ds(row0, S), :].rearrange("(c p) f -> p c f", p=P))
                )
                nc.sync.dma_start(
                    z5[0:1, :], _r(te[ds(row0, S), 0:1].rearrange("b one -> one b"))
                )
                tnat = sp.tile([P, 4], F32, tag="tnat")
                nc.sync.dma_start(
                    tnat[:], te[ds(row0, S), 0].rearrange("(c p) -> p c", p=P)
                )

                # ---- transpose inputs into zT chunks ----
                zT = zp.tile([P, 4, S], F32R, tag="zT")
                for k in range(4):
                    src = nat0 if k < 2 else nat1
                    fc = k % 2
                    ps = tpp.tile([P, S], F32, tag="tps", bufs=1)
                    for c in range(4):
                        nc.tensor.transpose(
                            _r(ps[:, ds(c * P, P)]),
                            _r(src[:, c, ds(fc * P, P)]),
                            ident_r[:],
                        )
                    nc.vector.tensor_copy(zT[:, k, :], ps[:])

                # ---- layer 1 ----
                h1 = hp.tile([P, HC, S], F32R, tag="hA")
                dh1 = hp.tile([P, HC, S], F32R, tag="dhA")
                for m in range(HC):
                    psf = mmp.tile([P, S], F32, tag="mmf")
                    for k in range(4):
                        nc.tensor.matmul(
                            psf[:],
                            w1s[:, k, ds(m * P, P)],
                            zT[:, k, :],
                            start=(k == 0),
                            stop=False,
                        )
                    nc.tensor.matmul(
                        psf[:],
                        w15[:, ds(m * P, P)],
                        z5[:],
                        start=False,
                        stop=True,
                    )
                    nc.scalar.activation(
                        h1[:, m, :], psf[:], RELU, bias=b1p[:, m : m + 1]
                    )
                    nc.vector.tensor_scalar(
                        dh1[:, m, :], h1[:, m, :], 0.0, w1rp[:, m : m + 1], GT, MULT
                    )

                # ---- layers 2 and 3 ----
                hprev, dhprev = h1, dh1
                for li, (ws, bp) in enumerate(((w2s, b2p), (w3s, b3p))):
                    hn = hp.tile([P, HC, S], F32R, tag="hB" if li == 0 else "hA")
                    dhn = hp.tile([P, HC, S], F32R, tag="dhB" if li == 0 else "dhA")
                    for m in range(HC):
                        psf = mmp.tile([P, S], F32, tag="mmf")
                        pst = mmp.tile([P, S], F32, tag="mmt")
                        for k in range(HC):
                            nc.tensor.matmul(
                                psf[:],
                                ws[:, k, ds(m * P, P)],
                                hprev[:, k, :],
                                start=(k == 0),
                                stop=(k == HC - 1),
                            )
                            nc.tensor.matmul(
                                pst[:],
                                ws[:, k, ds(m * P, P)],
                                dhprev[:, k, :],
                                start=(k == 0),
                                stop=(k == HC - 1),
                            )
                        # relu epilogue on DVE: (psum + b) max 0
                        nc.vector.tensor_scalar(
                            hn[:, m, :], psf[:], bp[:, m : m + 1], 0.0, ADD, MAX
                        )
                        # tangent mask: sign(h) in {0,1} on ACT, then mult on DVE
                        msk = sp.tile([P, S], F32, tag="mask", bufs=1)
                        nc.scalar.activation(msk[:], hn[:, m, :], SIGN)
                        nc.vector.tensor_tensor(dhn[:, m, :], msk[:], pst[:], MULT)
                    hprev, dhprev = hn, dhn

                # ---- layer 4 (no relu) ----
                fnnT = fp.tile([P, DC, S], F32R, tag="fnnT")
                dfnnT = fp.tile([P, DC, S], F32R, tag="dfnnT")
                for m in range(DC):
                    psf = mmp.tile([P, S], F32, tag="mmf")
                    pst = mmp.tile([P, S], F32, tag="mmt")
                    for k in range(HC):
                        nc.tensor.matmul(
                            psf[:],
                            w4s[:, k, ds(m * P, P)],
                            hprev[:, k, :],
                            start=(k == 0),
                            stop=(k == HC - 1),
                        )
                        nc.tensor.matmul(
                            pst[:],
                            w4s[:, k, ds(m * P, P)],
                            dhprev[:, k, :],
                            start=(k == 0),
                            stop=(k == HC - 1),
                        )
                    nc.scalar.activation(
                        fnnT[:, m, :], psf[:], IDENT, bias=b4p[:, m : m + 1]
                    )
                    nc.scalar.copy(dfnnT[:, m, :], pst[:])

                # ---- per-stripe t-derived scalars ----
                tsq = sp.tile([P, 4], F32, tag="tsq")
                nc.vector.tensor_tensor(tsq[:], tnat[:], tnat[:], MULT)
                a_ = sp.tile([P, 4], F32, tag="a_")
                nc.vector.tensor_tensor(a_[:], tnat[:], tsq[:], SUB)
                omt = sp.tile([P, 4], F32, tag="omt")
                nc.vector.tensor_scalar(omt[:], tnat[:], -1.0, 1.0, MULT, ADD)
                om2t = sp.tile([P, 4], F32, tag="om2t")
                nc.vector.tensor_scalar(om2t[:], tnat[:], -2.0, 1.0, MULT, ADD)

                # ---- transpose fnn/dfnn back to natural + combine ----
                dt_nat = fp.tile([P, 4, D], F32, tag="dt_nat")
                xt_nat = fp.tile([P, 4, D], F32, tag="xt_nat")
                for cp_i in range(2):
                    psF = tpp.tile([P, 2, D], F32, tag="ops", bufs=3)
                    psD = tpp.tile([P, 2, D], F32, tag="ops", bufs=3)
                    for ci in range(2):
                        c = 2 * cp_i + ci
                        for fc in range(DC):
                            nc.tensor.transpose(
                                _r(psF[:, ci, ds(fc * P, P)]),
                                fnnT[:, fc, ds(c * P, P)],
                                ident_r[:],
                            )
                            nc.tensor.transpose(
                                _r(psD[:, ci, ds(fc * P, P)]),
                                dfnnT[:, fc, ds(c * P, P)],
                                ident_r[:],
                            )
                    for ci in range(2):
                        c = 2 * cp_i + ci
                        tc_ = tnat[:, c : c + 1]
                        # dt = x1 - x0 (must read x0 before xt overwrites nat0)
                        nc.vector.tensor_tensor(
                            dt_nat[:, c, :], nat1[:, c, :], nat0[:, c, :], SUB
                        )
                        tm1 = sp.tile([P, D], F32, tag="tmp")
                        nc.scalar.mul(tm1[:], nat1[:, c, :], tc_)  # t*x1
                        nc.vector.tensor_scalar(
                            xt_nat[:, c, :], nat0[:, c, :], omt[:, c : c + 1], None, MULT
                        )
                        tm2 = sp.tile([P, D], F32, tag="tmp")
                        nc.scalar.mul(tm2[:], psF[:, ci, :], a_[:, c : c + 1])
                        nc.vector.tensor_tensor(
                            xt_nat[:, c, :], xt_nat[:, c, :], tm1[:], ADD
                        )
                        nc.vector.tensor_tensor(
                            xt_nat[:, c, :], xt_nat[:, c, :], tm2[:], ADD
                        )
                        tm3 = sp.tile([P, D], F32, tag="tmp")
                        nc.scalar.mul(tm3[:], psF[:, ci, :], om2t[:, c : c + 1])
                        nc.vector.tensor_tensor(
                            dt_nat[:, c, :], dt_nat[:, c, :], tm3[:], ADD
                        )
                        tm4 = sp.tile([P, D], F32, tag="tmp")
                        nc.scalar.mul(tm4[:], psD[:, ci, :], a_[:, c : c + 1])
                        nc.vector.tensor_tensor(
                            dt_nat[:, c, :], dt_nat[:, c, :], tm4[:], ADD
                        )

                nc.sync.dma_start(
                    xte[ds(row0, S), :].rearrange("(c p) f -> p c f", p=P), xt_nat[:]
                )
                nc.sync.dma_start(
                    dte[ds(row0, S), :].rearrange("(c p) f -> p c f", p=P), dt_nat[:]
                )

    nc.compile()
    return nc


def _get_nc():
    global _nc_cache
    if _nc_cache is None:
        _nc_cache = build()
    return _nc_cache


def kernel(x0, x1, t, W1, b1, W2, b2, W3, b3, W4, b4, trace=False, **trace_kwargs):
    nc = _get_nc()
    reps = {
        "W1": np.ascontiguousarray(W1, np.float32),
        "b1": np.ascontiguousarray(b1, np.float32),
        "W2": np.ascontiguousarray(W2, np.float32),
        "b2": np.ascontiguousarray(b2, np.float32),
        "W3": np.ascontiguousarray(W3, np.float32),
        "b3": np.ascontiguousarray(b3, np.float32),
        "W4": np.ascontiguousarray(W4, np.float32),
        "b4": np.ascontiguousarray(b4, np.float32),
    }
    in_maps = []
    for c in range(NCORES):
        sl = slice(c * BL, (c + 1) * BL)
        in_maps.append(
            {
                "x0": np.ascontiguousarray(x0[sl], np.float32),
                "x1": np.ascontiguousarray(x1[sl], np.float32),
                "t": np.ascontiguousarray(t[sl], np.float32),
                **reps,
            }
        )
    res = run_bass_kernel_spmd(
        nc, in_maps, list(range(NCORES)), trace=trace, **trace_kwargs
    )
    xt = np.concatenate([res.results[c]["xt"] for c in range(NCORES)], axis=0)
    dt_xt = np.concatenate([res.results[c]["dt_xt"] for c in range(NCORES)], axis=0)
    if trace:
        kernel.last_result = res
    return (xt, dt_xt)
